# revision 1
# baseline (speedup 1.0000x reference)
"""Trainium2 Bass kernel for nn_AttentionBlock (GroupNorm + 8-head self-attention
+ projection + residual) on input x:(8,512,32,32) f32.

Strategy: pure data-parallel over batch - each of the 8 NeuronCores processes
one batch element end-to-end (no collectives). Per core:

  x (512,1024) --GroupNorm--> xn --qkv matmul--> Q,K (o-part,t-free), V^T (s-part,c-free)
  per head h: S^T = K_h^T (Q_h + bq_h)  (s-part, t-free); expS = exp(S^T)
              Only Q's bias is applied: K's bias contributes a per-query
              constant that cancels in the softmax normalizer, and V's bias
              commutes with the softmax average (weights sum to 1) so it is
              folded into the projection bias on the host.
              H_ext = [V_h^T | 1]^T expS  (rows 0..63 = unnormalized AV, row 64 = denom)
              H = H_ext[0:64] * recip(H_ext[64]) broadcast (gpsimd partition_broadcast)
  out = proj @ H + (bproj + proj @ bv) + x

All big matmuls run in bf16 (full PE rate; ~2.5e-3 absmax error vs the fp32
reference, reference absmax ~5.4). Softmax scale and the per-head q/k/v row
split are folded into the qkv weights on the host.

Engine budget per iteration (CoreSim): PE ~82us is the floor (zero idle gaps
in steady state); ACT does only exp plus the Square-accumulate GroupNorm
sum-of-squares (both live in the Exp activation table - no table reloads);
the GroupNorm inv_std is a magic-constant rsqrt + 2 Newton steps on DVE ALU;
DVE does the PSUM->SBUF moves and normalize muls; Pool (gpsimd) takes the
partition broadcasts, hstg partition-shift DMAs and half the output stores.
The emission is software-pipelined one iteration ahead (loads+stats after
pair 0, normalize after pair 2, next QK prologue after pair 3), with the
prologue/projection PSUM rings split so the iteration head never waits on
the previous tail's drain. S matmuls alternate head halves so LDWEIGHTS
always targets the idle PE row groups.
"""

import numpy as np

import concourse.bacc as bacc
import concourse.bass2jax as bass2jax
import concourse.mybir as mybir
import concourse.tile as tile
from concourse.bass_utils import run_bass_kernel_spmd


def _install_neff_disk_cache():
    """Wrap compile_bir_kernel (as referenced by bass2jax's neuronx_cc hook)
    with a content-addressed on-disk cache keyed on the BIR JSON bytes, which
    are deterministic across processes - so repeated processes skip the
    walrus compile of an identical kernel."""
    if getattr(bass2jax, "_ant_neff_disk_cache", False):
        return
    import hashlib
    import os

    cache_dir = os.environ.get("BASS_NEFF_CACHE", "/tmp/bass_neff_cache")
    try:
        os.makedirs(cache_dir, exist_ok=True)
    except OSError:
        return
    orig = bass2jax.compile_bir_kernel

    def cached_compile(bir_json, tmpdir, neff_name="file.neff"):
        key = hashlib.sha256(bytes(bir_json)).hexdigest()
        path = os.path.join(cache_dir, key + ".neff")
        out_path = os.path.join(tmpdir, neff_name)
        if os.path.exists(path):
            import shutil

            shutil.copyfile(path, out_path)
            return out_path
        r = orig(bir_json, tmpdir, neff_name=neff_name)
        try:
            tmp = path + f".tmp{os.getpid()}"
            with open(r, "rb") as f:
                data = f.read()
            with open(tmp, "wb") as f:
                f.write(data)
            os.replace(tmp, path)
        except Exception:
            pass
        return r

    bass2jax.compile_bir_kernel = cached_compile
    bass2jax._ant_neff_disk_cache = True


_install_neff_disk_cache()

# A/B bisect knobs (timing experiments; default all-off = production)
AB = {}

B = 8
C = 512
T = 1024
HEADS = 8
HD = 64  # head dim
G = 32  # groupnorm groups
GSIZE = C // G  # 16 channels per group
EPS = 1e-5

F32 = mybir.dt.float32
BF16 = mybir.dt.bfloat16
I32 = mybir.dt.int32
AX = mybir.AxisListType
ALU = mybir.AluOpType
ACTF = mybir.ActivationFunctionType

# consts layout (per 128-channel chunk j): [gnw, gnb, bprojK, gmat(32)]
NCONST = 35


def _emit_weights(nc, pp, dram):
    """Iteration-invariant weight/constant loads (emitted once; the repeated
    timing bodies keep them resident in SBUF, as a deployment would)."""
    wqkvT_r = dram["wqkvT"].rearrange("(j p) o -> j p o", p=128)
    wprojT_r = dram["wprojT"].rearrange("(j p) o -> j p o", p=128)
    w = {}
    wqkvT = []
    for j in range(4):
        wq_sb = pp.tile([128, 3 * C], BF16, name=f"wqkvT{j}", tag=f"wqkvT{j}")
        nc.sync.dma_start(out=wq_sb, in_=wqkvT_r[j])
        wqkvT.append(wq_sb)
    wprojT = []
    for j in range(4):
        wp_sb = pp.tile([128, C], BF16, name=f"wprojT{j}", tag=f"wprojT{j}")
        nc.gpsimd.dma_start(out=wp_sb, in_=wprojT_r[j])
        wprojT.append(wp_sb)
    consts = pp.tile([128, 4, NCONST], F32, name="consts", tag="consts")
    nc.sync.dma_start(out=consts, in_=dram["consts"])
    gmatT = pp.tile([G, 4, 128], F32, name="gmatT", tag="gmatT")
    nc.sync.dma_start(out=gmatT, in_=dram["gmatT"])
    bq = pp.tile([128, 4], F32, name="bq", tag="bq")
    nc.gpsimd.dma_start(out=bq, in_=dram["bq"].rearrange("(j p) o -> p (j o)", p=128))
    w.update(wqkvT=wqkvT, wprojT=wprojT, consts=consts, gmatT=gmatT, bq=bq)
    return w


def _emit_front_loads(nc, pp, wp, pool_ps, dram, w):
    """x loads + GroupNorm statistics for one iteration (emitted one stage
    ahead, mid-way through the previous iteration's attention). x is spread
    over the SP/DVE/ACT DMA queues so no single ring carries more than
    ~1 MB per iteration."""
    x_r = dram["x"].rearrange("(j p) t -> j p t", p=128)

    fr = dict(w)
    x_q = [nc.sync, nc.sync, nc.scalar, nc.scalar]
    xt = []
    for j in range(4):
        x_sb = pp.tile([128, T], F32, name=f"x{j}", tag=f"x{j}", bufs=2)
        x_q[j].dma_start(out=x_sb, in_=x_r[j])
        xt.append(x_sb)

    # Sum(x) on DVE, Sum(x^2) on ACT (Square + accumulator; Square lives in
    # the Exp table so the ACT engine keeps a single table all kernel).
    stats = []
    for j in range(4):
        stat = pp.tile([128, 2], F32, name=f"stat{j}", tag=f"stat{j}", bufs=2)
        nc.vector.reduce_sum(stat[:, 0:1], xt[j], axis=AX.X)
        if AB.get("stat_dve"):
            scr = wp.tile([128, T], F32, name="sqscr", tag="oto", bufs=2)
            nc.vector.scalar_tensor_tensor(
                out=scr, in0=xt[j], scalar=1.0, in1=xt[j],
                op0=ALU.mult, op1=ALU.mult, accum_out=stat[:, 1:2],
            )
        else:
            sqd = wp.tile([128, T], BF16, name="sqd", tag="sqd", bufs=1)
            nc.scalar.activation(
                out=sqd, in_=xt[j], func=ACTF.Square, accum_out=stat[:, 1:2]
            )
        stats.append(stat)

    fr.update(xt=xt, stats=stats)
    return fr


def _emit_front_norm(nc, pp, wp, pool_ps, fr):
    """GroupNorm normalization chain + xn for a front started by
    _emit_front_loads."""
    consts, gmatT, stats, xt = fr["consts"], fr["gmatT"], fr["stats"], fr["xt"]
    gnw = [consts[:, j, 0:1] for j in range(4)]
    gnb = [consts[:, j, 1:2] for j in range(4)]
    gmat = [consts[:, j, 3 : 3 + G] for j in range(4)]

    gsum = pool_ps.tile([G, 2], F32, name="gsum", tag="sm", bufs=4 if AB.get("psum_orig") else 2)
    for j in range(4):
        nc.tensor.matmul(
            out=gsum, lhsT=gmat[j], rhs=stats[j], start=(j == 0), stop=(j == 3)
        )
    gstat = pp.tile([G, 2], F32, name="gstat", tag="gstat", bufs=2)
    nc.vector.tensor_scalar_mul(gstat, gsum, 1.0 / float(GSIZE * T))
    m2 = pp.tile([G, 1], F32, name="m2", tag="m2", bufs=2)
    nc.vector.tensor_tensor(out=m2, in0=gstat[:, 0:1], in1=gstat[:, 0:1], op=ALU.mult)
    var = pp.tile([G, 1], F32, name="var", tag="var", bufs=2)
    nc.vector.tensor_tensor(out=var, in0=gstat[:, 1:2], in1=m2, op=ALU.subtract)
    nc.vector.tensor_scalar_add(var, var, EPS)
    # inv_std = rsqrt(var) via the magic-constant seed + 2 Newton steps, all
    # on DVE int/float ALU ops ([G,1] tiles - sub-100ns each). Keeps the ACT
    # engine exp-only (no per-iteration activation-table reloads).
    ish = pp.tile([G, 1], I32, name="ish", tag="ish", bufs=2)
    nc.vector.tensor_scalar(
        out=ish,
        in0=var.bitcast(I32),
        scalar1=1,
        scalar2=None,
        op0=ALU.logical_shift_right,
    )
    imag = pp.tile([G, 1], I32, name="imag", tag="imag", bufs=2)
    nc.vector.tensor_scalar(
        out=imag, in0=ish, scalar1=-1, scalar2=0x5F3759DF, op0=ALU.mult, op1=ALU.add
    )
    y = imag.bitcast(F32)
    grs = pp.tile([G, 2], F32, name="grs", tag="grs", bufs=2)
    nc.vector.tensor_copy(out=grs[:, 0:1], in_=gstat[:, 0:1])
    for it in range(2):
        vy = wp.tile([G, 1], F32, name="vy", tag="vy", bufs=2)
        nc.vector.tensor_tensor(out=vy, in0=var, in1=y, op=ALU.mult)
        vyy = wp.tile([G, 1], F32, name="vyy", tag="vyy", bufs=2)
        nc.vector.tensor_tensor(out=vyy, in0=vy, in1=y, op=ALU.mult)
        w_t = wp.tile([G, 1], F32, name="wns", tag="wns", bufs=2)
        nc.vector.tensor_scalar(
            out=w_t, in0=vyy, scalar1=-0.5, scalar2=1.5, op0=ALU.mult, op1=ALU.add
        )
        dst = grs[:, 1:2] if it == 1 else pp.tile([G, 1], F32, name="y1", tag="y1", bufs=2)
        nc.vector.tensor_tensor(out=dst, in0=y, in1=w_t, op=ALU.mult)
        y = dst

    xn = []
    for j in range(4):
        chs = pool_ps.tile([128, 2], F32, name=f"chs{j}", tag="sm", bufs=4 if AB.get("psum_orig") else 2)
        nc.tensor.matmul(out=chs, lhsT=gmatT[:, j], rhs=grs, start=True, stop=True)
        a_j = pp.tile([128, 1], F32, name=f"a{j}", tag=f"a{j}", bufs=2)
        nc.vector.tensor_tensor(out=a_j, in0=gnw[j], in1=chs[:, 1:2], op=ALU.mult)
        nb = wp.tile([128, 1], F32, name="nb", tag="nb")
        nc.vector.tensor_tensor(out=nb, in0=chs[:, 0:1], in1=a_j, op=ALU.mult)
        b_j = pp.tile([128, 1], F32, name=f"b{j}", tag=f"b{j}", bufs=2)
        nc.vector.tensor_tensor(out=b_j, in0=gnb[j], in1=nb, op=ALU.subtract)
        xn_j = pp.tile([128, T], BF16, name=f"xn{j}", tag=f"xn{j}", bufs=2)
        nc.vector.tensor_scalar(
            out=xn_j, in0=xt[j], scalar1=a_j, scalar2=b_j, op0=ALU.mult, op1=ALU.add
        )
        xn.append(xn_j)
    fr["xn"] = xn


def _emit_prologue(nc, pp, wp, pool_ps, fr):
    """Pair-0 QK + r for front `fr` (hoisted into the previous body so the
    first S matmuls of the next iteration are ready the moment its body
    starts)."""
    xn, wqkvT, bq = fr["xn"], fr["wqkvT"], fr["bq"]
    q0 = pp.tile([128, T], BF16, name="q0", tag="q0")
    k0 = pp.tile([128, T], BF16, name="k0", tag="k0")
    for grp in range(4):
        which, tb = grp // 2, grp % 2
        col0 = which * C
        ps = pool_ps.tile([128, 512], F32, name="qkps", tag="sm", bufs=4 if AB.get("psum_orig") else 2)
        for c in range(4):
            nc.tensor.matmul(
                out=ps,
                lhsT=wqkvT[c][:, col0 : col0 + 128],
                rhs=xn[c][:, tb * 512 : (tb + 1) * 512],
                start=(c == 0),
                stop=(c == 3),
            )
        if which == 0:
            nc.vector.tensor_scalar_add(
                q0[:, tb * 512 : (tb + 1) * 512], ps, bq[:, 0:1]
            )
        else:
            nc.vector.tensor_copy(out=k0[:, tb * 512 : (tb + 1) * 512], in_=ps)
    return {"q0": q0, "k0": k0}


def _emit_attn(nc, pp, wp, pool_ps, dram, fr, pro, hook1=None, hook2=None, hook3=None):
    """Attention + projection + residual for a prepared front `fr` whose
    pair-0 QK prologue `pro` was already emitted.

    hook1/hook2 are invoked after the pair-0 / pair-2 phases to emit the next
    iteration's loads+stats and normalization; hook3 after the pair-3 S loop
    to emit the next iteration's prologue - so every engine sees the next
    front's work well before this iteration's tail drains."""
    out_r = dram["out"].rearrange("(j p) t -> j p t", p=128)
    xt, xn, wqkvT, wprojT, bq = fr["xt"], fr["xn"], fr["wqkvT"], fr["wprojT"], fr["bq"]
    consts = fr["consts"]
    bproj = [consts[:, j, 2:3] for j in range(4)]

    q_sb = [pro["q0"], None, None, None]
    k_sb = [pro["k0"], None, None, None]
    hn_sb = []
    for j in range(4):
        hn_j = pp.tile([128, T], BF16, name=f"hn{j}", tag=f"hn{j}")
        hn_sb.append(hn_j)
    vT = [None] * 8

    def emit_vt_chunk(s: int) -> None:
        """V^T s-tile: V^T[s,:] for all heads (+ ones col), 4 matmuls."""
        vt_s = pp.tile([128, HEADS, HD + 1], BF16, name=f"vT{s}", tag=f"vT{s}")
        nc.gpsimd.memset(vt_s[:, :, HD : HD + 1], 1.0)
        vps = pool_ps.tile([128, C], F32, name=f"vps{s}", tag="sm", bufs=4 if AB.get("psum_orig") else 2)
        for c in range(4):
            nc.tensor.matmul(
                out=vps,
                lhsT=xn[c][:, s * 128 : (s + 1) * 128],
                rhs=wqkvT[c][:, 2 * C : 3 * C],
                start=(c == 0),
                stop=(c == 3),
            )
        nc.vector.tensor_copy(
            out=vt_s[:, :, 0:HD], in_=vps.rearrange("p (h d) -> p h d", d=HD)
        )
        vT[s] = vt_s

    def make_qk_chunks(jt: int):
        """QK o-tile pair jt as 8 chunks of 2 matmuls (4 groups x 4 c-mms)."""
        dsts = {}
        for which in range(2):
            dsts[which] = pp.tile(
                [128, T], BF16, name=f"{'qk'[which]}{jt}", tag=f"{'qk'[which]}{jt}"
            )
        state = {}

        def chunk(s: int) -> None:
            grp = s // 2  # 0..3: (which, tb)
            which, tb = grp // 2, grp % 2
            col0 = which * C
            if s % 2 == 0:
                state["ps"] = pool_ps.tile([128, 512], F32, name="qkps", tag="sm", bufs=4 if AB.get("psum_orig") else 2)
            ps = state["ps"]
            for c in (2 * (s % 2), 2 * (s % 2) + 1):
                nc.tensor.matmul(
                    out=ps,
                    lhsT=wqkvT[c][:, col0 + jt * 128 : col0 + (jt + 1) * 128],
                    rhs=xn[c][:, tb * 512 : (tb + 1) * 512],
                    start=(c == 0),
                    stop=(c == 3),
                )
            if s % 2 == 1:
                if which == 0:
                    nc.vector.tensor_scalar_add(
                        dsts[0][:, tb * 512 : (tb + 1) * 512], ps, bq[:, jt : jt + 1]
                    )
                else:
                    nc.vector.tensor_copy(
                        out=dsts[1][:, tb * 512 : (tb + 1) * 512], in_=ps
                    )

        def finish():
            q_sb[jt] = dsts[0]
            k_sb[jt] = dsts[1]

        return chunk, finish

    def emit_s_exp(p: int, s: int, expS) -> None:
        """S^T matmuls + exp for head pair p, s-block s (both heads).

        The four matmuls alternate head halves (row groups 0-1 vs 2-3) so
        each LDWEIGHTS targets the array half the in-flight matmul is not
        using and can always be pulled ahead."""
        jt = p
        sps = {}
        for hh in range(2):
            sps[hh] = pool_ps.tile([128, T], F32, name="sps", tag="st", bufs=2)
        order = (
            [(0, 0), (1, 0), (0, 1), (1, 1)]
            if AB.get("s_alt", True)
            else [(0, 0), (0, 1), (1, 0), (1, 1)]
        )
        for hh, tb in order:
            off = 64 * hh
            nc.tensor.matmul(
                out=sps[hh][:, tb * 512 : (tb + 1) * 512],
                lhsT=k_sb[jt][off : off + 64, s * 128 : (s + 1) * 128],
                rhs=q_sb[jt][off : off + 64, tb * 512 : (tb + 1) * 512],
                start=True,
                stop=True,
            )
        for hh in range(2):
            es = wp.tile([128, T], BF16, name="expS", tag="expS", bufs=40)
            nc.scalar.activation(
                out=es,
                in_=sps[hh],
                func=ACTF.Identity if AB.get("exp_copy") else ACTF.Exp,
            )
            expS[hh].append(es)

    def make_av_chunks(p: int, expS):
        """AV + normalize for head pair p as 8 chunks of 4 matmuls."""
        jt = p
        state = {}

        def chunk(s: int) -> None:
            grp = s // 2  # (hh, tb)
            hh, tb = grp // 2, grp % 2
            h = 2 * p + hh
            if s % 2 == 0:
                state["ps"] = pool_ps.tile(
                    [HD + 1, 512], F32,
                    name="hps",
                    tag="sm" if AB.get("psum_orig") else "hp",
                    bufs=4 if AB.get("psum_orig") else 2,
                )
            hps = state["ps"]
            s0 = 4 * (s % 2)
            for si in range(s0, s0 + 4):
                nc.tensor.matmul(
                    out=hps,
                    lhsT=vT[si][:, h, :],
                    rhs=expS[hh][si][:, tb * 512 : (tb + 1) * 512],
                    start=(si == 0),
                    stop=(si == 7),
                )
            if s % 2 == 1:
                off = 64 * hh
                rrow = wp.tile([1, 512], F32, name="rrow", tag="rrow", bufs=2)
                nc.vector.reciprocal(out=rrow, in_=hps[HD : HD + 1, :])
                rb = wp.tile([64, 512], F32, name="rb", tag="rb", bufs=2)
                nc.gpsimd.partition_broadcast(out_ap=rb, in_ap=rrow, channels=64)
                if off == 0:
                    nc.vector.tensor_tensor(
                        out=hn_sb[jt][0:64, tb * 512 : (tb + 1) * 512],
                        in0=hps[0:HD, :],
                        in1=rb,
                        op=ALU.mult,
                    )
                else:
                    hstg = wp.tile([64, 512], BF16, name="hstg", tag="hstg", bufs=2)
                    nc.vector.tensor_tensor(
                        out=hstg, in0=hps[0:HD, :], in1=rb, op=ALU.mult
                    )
                    if not AB.get("no_hstg"):
                        (nc.sync if AB.get("store_sync") else nc.gpsimd).dma_start(
                            out=hn_sb[jt][64:128, tb * 512 : (tb + 1) * 512], in_=hstg
                        )

        return chunk

    av_chunk = None
    for p in range(4):
        expS = {0: [], 1: []}
        if p < 3:
            qk_chunk, qk_finish = make_qk_chunks(p + 1)
        else:
            qk_chunk, qk_finish = None, None
        for s in range(8):
            emit_s_exp(p, s, expS)
            if p == 0:
                emit_vt_chunk(s)
            if av_chunk is not None:
                av_chunk(s)
            if qk_chunk is not None:
                qk_chunk(s)
        if qk_finish is not None:
            qk_finish()
        av_chunk = make_av_chunks(p, expS)
        if p == 0 and hook1 is not None:
            hook1()
        if p == 2 and hook2 is not None:
            hook2()
    if hook3 is not None:
        hook3()
    for s in range(8):
        av_chunk(s)

    # ---- projection + bias + residual ----
    for o in range(4):
        oto = wp.tile([128, T], F32, name="oto", tag="oto", bufs=2)
        for tb in range(2):
            pps = pool_ps.tile([128, 512], F32, name="pps", tag="sm" if AB.get("psum_orig") else "hp", bufs=4 if AB.get("psum_orig") else 2)
            for c in range(4):
                nc.tensor.matmul(
                    out=pps,
                    lhsT=wprojT[c][:, o * 128 : (o + 1) * 128],
                    rhs=hn_sb[c][:, tb * 512 : (tb + 1) * 512],
                    start=(c == 0),
                    stop=(c == 3),
                )
            nc.vector.scalar_tensor_tensor(
                out=oto[:, tb * 512 : (tb + 1) * 512],
                in0=pps,
                scalar=bproj[o],
                in1=xt[o][:, tb * 512 : (tb + 1) * 512],
                op0=ALU.add,
                op1=ALU.add,
            )
        (nc.sync if (o < 2 or AB.get("store_sync")) else nc.gpsimd).dma_start(
            out=out_r[o], in_=oto
        )


def _emit_iters(nc, pp, wp, pool_ps, dram, repeats: int, w=None) -> None:
    if w is None:
        w = _emit_weights(nc, pp, dram)
    fr = _emit_front_loads(nc, pp, wp, pool_ps, dram, w)
    _emit_front_norm(nc, pp, wp, pool_ps, fr)
    pro = _emit_prologue(nc, pp, wp, pool_ps, fr)
    for i in range(repeats):
        nxt = {}
        if i < repeats - 1:
            def hook1():
                nxt["fr"] = _emit_front_loads(nc, pp, wp, pool_ps, dram, w)

            def hook2():
                _emit_front_norm(nc, pp, wp, pool_ps, nxt["fr"])

            def hook3():
                nxt["pro"] = _emit_prologue(nc, pp, wp, pool_ps, nxt["fr"])
        else:
            hook1 = hook2 = hook3 = None
        _emit_attn(nc, pp, wp, pool_ps, dram, fr, pro, hook1, hook2, hook3)
        if i < repeats - 1:
            fr, pro = nxt["fr"], nxt["pro"]


def _emit(nc, repeats: int = 1, loop_n: int | None = None) -> None:
    dram = {
        "x": nc.dram_tensor("x", [C, T], F32, kind="ExternalInput").ap(),
        "wqkvT": nc.dram_tensor("wqkvT", [C, 3 * C], BF16, kind="ExternalInput").ap(),
        "bq": nc.dram_tensor("bq", [C, 1], F32, kind="ExternalInput").ap(),
        "wprojT": nc.dram_tensor("wprojT", [C, C], BF16, kind="ExternalInput").ap(),
        "consts": nc.dram_tensor(
            "consts", [128, 4, NCONST], F32, kind="ExternalInput"
        ).ap(),
        "gmatT": nc.dram_tensor("gmatT", [G, 4, 128], F32, kind="ExternalInput").ap(),
        "out": nc.dram_tensor("out", [C, T], F32, kind="ExternalOutput").ap(),
    }
    with tile.TileContext(nc) as tc:
        with (
            tc.tile_pool(name="persist", bufs=1) as pp,
            tc.tile_pool(name="work", bufs=2) as wp,
            tc.tile_pool(name="psum", bufs=1, space="PSUM") as pool_ps,
        ):
            if loop_n is not None:
                w = _emit_weights(nc, pp, dram)
                with tc.For_i(0, loop_n) as _i:
                    _emit_iters(nc, pp, wp, pool_ps, dram, repeats, w=w)
            else:
                _emit_iters(nc, pp, wp, pool_ps, dram, repeats)


_NC_CACHE = {}


def build_nc(repeats: int = 1, loop_n: int | None = None):
    key = (repeats, loop_n, tuple(sorted(AB.items())))
    if key not in _NC_CACHE:
        nc = bacc.Bacc("TRN2", target_bir_lowering=False, debug=False, num_devices=B)
        _emit(nc, repeats=repeats, loop_n=loop_n)
        nc.compile()
        _NC_CACHE[key] = nc
    return _NC_CACHE[key]


def prep_inputs(x, gn_w, gn_b, qkv_w, qkv_b, proj_w, proj_b):
    """Host-side reformat: returns the per-core in_map dicts (core i = batch i)."""
    import ml_dtypes

    x = np.ascontiguousarray(np.asarray(x, dtype=np.float32))
    gn_w = np.asarray(gn_w, dtype=np.float32)
    gn_b = np.asarray(gn_b, dtype=np.float32)
    qkv_w = np.asarray(qkv_w, dtype=np.float32)
    qkv_b = np.asarray(qkv_b, dtype=np.float32)
    proj_w = np.asarray(proj_w, dtype=np.float32)
    proj_b = np.asarray(proj_b, dtype=np.float32)

    scale = float(HD) ** -0.25
    idx_q = np.concatenate([np.arange(3 * HD * h, 3 * HD * h + HD) for h in range(HEADS)])
    idx_k = idx_q + HD
    idx_v = idx_q + 2 * HD
    wq = qkv_w[idx_q] * scale
    wk = qkv_w[idx_k] * scale
    wv = qkv_w[idx_v]
    wqkvT = np.ascontiguousarray(
        np.concatenate([wq, wk, wv], axis=0).T.astype(ml_dtypes.bfloat16)
    )  # (512, 1536) bf16
    wprojT = np.ascontiguousarray(proj_w.T.astype(ml_dtypes.bfloat16))

    # Softmax bias algebra: K's bias contributes a per-query constant that
    # cancels in the softmax normalizer, and V's bias commutes with the
    # softmax average (weights sum to 1) -> only Q's bias is applied (on the
    # q tiles), K/V biases are dropped/folded into the projection bias.
    bq = (qkv_b[idx_q] * scale).reshape(C, 1)
    bprojK = proj_b + proj_w @ qkv_b[idx_v]

    consts = np.zeros((128, 4, NCONST), dtype=np.float32)
    gmatT = np.zeros((G, 4, 128), dtype=np.float32)
    for j in range(4):
        consts[:, j, 0] = gn_w[j * 128 : (j + 1) * 128]
        consts[:, j, 1] = gn_b[j * 128 : (j + 1) * 128]
        consts[:, j, 2] = bprojK[j * 128 : (j + 1) * 128]
        for cl in range(128):
            g = 8 * j + cl // GSIZE
            consts[cl, j, 3 + g] = 1.0  # gmat one-hot [128, G]
            gmatT[g, j, cl] = 1.0

    shared = {
        "wqkvT": wqkvT,
        "bq": np.ascontiguousarray(bq),
        "wprojT": wprojT,
        "consts": consts,
        "gmatT": gmatT,
    }
    in_maps = []
    for b in range(B):
        m = {"x": np.ascontiguousarray(x[b].reshape(C, T))}
        m.update(shared)
        in_maps.append(m)
    return in_maps


def kernel(x, gn_w, gn_b, qkv_w, qkv_b, proj_w, proj_b):
    import os

    # The axon client has no NTFF hook; a stray BASS_TRACE=1 would crash the
    # trace path inside run_bass_kernel_spmd.
    os.environ.setdefault("BASS_NEVER_TRACE", "1")
    in_maps = prep_inputs(x, gn_w, gn_b, qkv_w, qkv_b, proj_w, proj_b)
    nc = build_nc()
    res = run_bass_kernel_spmd(nc, in_maps, core_ids=list(range(B)))
    out = np.stack([res.results[i]["out"] for i in range(B)], axis=0)
    return out.reshape(B, C, 32, 32).astype(np.float32)



# revision 11
# speedup vs baseline: 1.0485x; 1.0485x over previous
"""Trainium2 Bass kernel for nn_AttentionBlock (GroupNorm + 8-head self-attention
+ projection + residual) on input x:(8,512,32,32) f32.

Strategy: pure data-parallel over batch - each of the 8 NeuronCores processes
one batch element end-to-end (no collectives). Per core:

  x (512,1024) --GroupNorm--> xn (bf16 + fp8 pair copies) --> Q,K via bf16
  matmul (o-part/t-free), V^T via fp8 DoubleRow (s-part/c-free)
  per head h: S^T = K_h^T (Q_h + bq_h)  (bf16 PE, s-part, t-free)
              expS = exp(S^T - 3) in fp8 (the -3 shift cancels in the
              softmax normalizer; it keeps exp output centered in e4m3
              range). Only Q's bias is applied: K's bias cancels in the
              softmax, V's bias is folded into the residual on the host.
              H_ext = [V_h^T | 1]^T expS via fp8 DoubleRow (K=256 per mm)
              H = H_ext[0:64] * recip(H_ext[64]) (gpsimd partition_broadcast)
  out = (proj64 @ H)/64 + (x + bproj')   (bproj' = bproj + proj@bv, on-device)

The V^T / AV / projection matmuls run in fp8e4 (e4m3) with DoubleRow perf
mode: 2 contraction rows per PE cell, so one matmul contracts K=256 at half
the PE streaming cycles of bf16. Q/K stay bf16 end-to-end: fp8 quantization
of xn/Wq/Wk adds ~5% logit noise which lands the final error at ~1.9e-2 -
too close to the gate - while fp8 on the value path (V, expS, hn, proj)
washes out in the softmax average. fp8 weights are pre-scaled x64 on the
host so their sigma~1 lands mid-e4m3; the 1/64 is folded into the existing
PSUM->SBUF copies (free). S matmuls (K=64) stay bf16; the two heads' S
matmuls land on disjoint PE row halves (auto tile_position) and overlap on
hardware.

Engine budget per iteration (TimelineSim): ACT ~64us of exp is the floor
(8 heads x 1024^2 logits / 128 lanes / 1.2GHz); PE ~41us (27 of it the bf16
S matmuls); DVE ~50us. GroupNorm statistics run on DVE (sum) + DVE
scalar_tensor_tensor (sum of squares) to keep ACT exp-only; inv_std is a
magic-constant rsqrt + 2 Newton steps on DVE ALU. The emission is
software-pipelined one iteration ahead (loads+stats after pair 0, normalize
after pair 2, next QK prologue after pair 3), with prologue/projection PSUM
rings split so the iteration head never waits on the previous tail's drain.
"""

import numpy as np

import concourse.bacc as bacc
import concourse.bass2jax as bass2jax
import concourse.mybir as mybir
import concourse.tile as tile
from concourse.bass_utils import run_bass_kernel_spmd


def _install_neff_disk_cache():
    """Wrap compile_bir_kernel (as referenced by bass2jax's neuronx_cc hook)
    with a content-addressed on-disk cache keyed on the BIR JSON bytes, which
    are deterministic across processes - so repeated processes skip the
    walrus compile of an identical kernel."""
    if getattr(bass2jax, "_ant_neff_disk_cache", False):
        return
    import hashlib
    import os

    cache_dir = os.environ.get("BASS_NEFF_CACHE", "/tmp/bass_neff_cache")
    try:
        os.makedirs(cache_dir, exist_ok=True)
    except OSError:
        return
    orig = bass2jax.compile_bir_kernel

    def cached_compile(bir_json, tmpdir, neff_name="file.neff"):
        key = hashlib.sha256(bytes(bir_json)).hexdigest()
        path = os.path.join(cache_dir, key + ".neff")
        out_path = os.path.join(tmpdir, neff_name)
        if os.path.exists(path):
            import shutil

            shutil.copyfile(path, out_path)
            return out_path
        r = orig(bir_json, tmpdir, neff_name=neff_name)
        try:
            tmp = path + f".tmp{os.getpid()}"
            with open(r, "rb") as f:
                data = f.read()
            with open(tmp, "wb") as f:
                f.write(data)
            os.replace(tmp, path)
        except Exception:
            pass
        return r

    bass2jax.compile_bir_kernel = cached_compile
    bass2jax._ant_neff_disk_cache = True


_install_neff_disk_cache()

# A/B bisect knobs (timing experiments; default all-off = production)
AB = {}

B = 8
C = 512
T = 1024
HEADS = 8
HD = 64  # head dim
HDP = 80  # padded per-head slot in vT tiles (16B-aligned base per head)
G = 32  # groupnorm groups
GSIZE = C // G  # 16 channels per group
EPS = 1e-5
WSCALE = 64.0  # host premultiplier on fp8 weights (keeps sigma ~1 in e4m3)
WINV = 1.0 / WSCALE
EXP_BIAS = -3.0  # exp(S-3): cancels in softmax, centers e4m3 range

F32 = mybir.dt.float32
BF16 = mybir.dt.bfloat16
F8 = mybir.dt.float8e4
I32 = mybir.dt.int32
AX = mybir.AxisListType
ALU = mybir.AluOpType
ACTF = mybir.ActivationFunctionType
DR = mybir.MatmulPerfMode.DoubleRow

# consts layout (per 128-channel chunk j): [gnw, gnb, bprojK, gmat(32)]
NCONST = 35


def _emit_weights(nc, pp, dram):
    """Iteration-invariant weight/constant loads (emitted once; the repeated
    timing bodies keep them resident in SBUF, as a deployment would)."""
    w = {}
    wqkT_r = dram["wqkT"].rearrange("(j p) o -> j p o", p=128)
    wqkT = []
    for j in range(4):
        t = pp.tile([128, 2 * C], BF16, name=f"wqkT{j}", tag=f"wqkT{j}")
        nc.sync.dma_start(out=t, in_=wqkT_r[j])
        wqkT.append(t)
    if AB.get("v_bf16"):
        wvT_r = dram["wvT"].rearrange("(j p) o -> j p o", p=128)
        wv8 = []
        for j in range(4):
            t = pp.tile([128, C], BF16, name=f"wvT{j}", tag=f"wvT{j}")
            nc.sync.dma_start(out=t, in_=wvT_r[j])
            wv8.append(t)
    else:
        wv8 = []
        for m in range(2):
            t = pp.tile([128, 2, C], F8, name=f"wv8_{m}", tag=f"wv8_{m}")
            nc.sync.dma_start(out=t, in_=dram["wv8"][m])
            wv8.append(t)
    if AB.get("proj_bf16"):
        wprojT_r = dram["wprojT"].rearrange("(j p) o -> j p o", p=128)
        wproj8 = []
        for j in range(4):
            t = pp.tile([128, C], BF16, name=f"wprojT{j}", tag=f"wprojT{j}")
            nc.gpsimd.dma_start(out=t, in_=wprojT_r[j])
            wproj8.append(t)
    else:
        wproj8 = []
        for m in range(2):
            t = pp.tile([128, 2, C], F8, name=f"wproj8_{m}", tag=f"wproj8_{m}")
            nc.gpsimd.dma_start(out=t, in_=dram["wproj8"][m])
            wproj8.append(t)
    consts = pp.tile([128, 4, NCONST], F32, name="consts", tag="consts")
    nc.sync.dma_start(out=consts, in_=dram["consts"])
    gmatT = pp.tile([G, 4, 128], F32, name="gmatT", tag="gmatT")
    nc.sync.dma_start(out=gmatT, in_=dram["gmatT"])
    bq = pp.tile([128, 4], F32, name="bq", tag="bq")
    nc.gpsimd.dma_start(out=bq, in_=dram["bq"].rearrange("(j p) o -> p (j o)", p=128))
    eb = pp.tile([128, 1], F32, name="expbias", tag="expbias")
    nc.gpsimd.memset(eb, EXP_BIAS)
    w.update(wqkT=wqkT, wv8=wv8, wproj8=wproj8, consts=consts, gmatT=gmatT, bq=bq, eb=eb)
    return w


def _emit_front_loads(nc, pp, wp, pool_ps, dram, w):
    """x loads + GroupNorm statistics for one iteration (emitted one stage
    ahead, mid-way through the previous iteration's attention). x is spread
    over the SP/DVE/ACT DMA queues so no single ring carries more than
    ~1 MB per iteration."""
    x_r = dram["x"].rearrange("(j p) t -> j p t", p=128)

    fr = dict(w)
    x_q = [nc.sync, nc.sync, nc.scalar, nc.scalar]
    xt = []
    for j in range(4):
        x_sb = pp.tile([128, T], F32, name=f"x{j}", tag=f"x{j}", bufs=2)
        x_q[j].dma_start(out=x_sb, in_=x_r[j])
        xt.append(x_sb)

    # Sum(x) on DVE; Sum(x^2) on DVE too (scalar_tensor_tensor with
    # accumulator) so the ACT engine stays exp-only.
    stats = []
    for j in range(4):
        stat = pp.tile([128, 2], F32, name=f"stat{j}", tag=f"stat{j}", bufs=2)
        nc.vector.reduce_sum(stat[:, 0:1], xt[j], axis=AX.X)
        if AB.get("stat_act"):
            sqd = wp.tile([128, T], BF16, name="sqd", tag="sqd", bufs=1)
            nc.scalar.activation(
                out=sqd, in_=xt[j], func=ACTF.Square, accum_out=stat[:, 1:2]
            )
        else:
            scr = wp.tile([128, T], F32, name="sqscr", tag="oto", bufs=2)
            nc.vector.scalar_tensor_tensor(
                out=scr, in0=xt[j], scalar=1.0, in1=xt[j],
                op0=ALU.mult, op1=ALU.mult, accum_out=stat[:, 1:2],
            )
        stats.append(stat)

    fr.update(xt=xt, stats=stats)
    return fr


def _emit_front_norm(nc, pp, wp, pool_ps, fr):
    """GroupNorm normalization chain + xn (fp8 DoubleRow pair tiles) for a
    front started by _emit_front_loads."""
    consts, gmatT, stats, xt = fr["consts"], fr["gmatT"], fr["stats"], fr["xt"]
    gnw = [consts[:, j, 0:1] for j in range(4)]
    gnb = [consts[:, j, 1:2] for j in range(4)]
    gmat = [consts[:, j, 3 : 3 + G] for j in range(4)]

    gsum = pool_ps.tile([G, 2], F32, name="gsum", tag="sm", bufs=2)
    for j in range(4):
        nc.tensor.matmul(
            out=gsum, lhsT=gmat[j], rhs=stats[j], start=(j == 0), stop=(j == 3)
        )
    gstat = pp.tile([G, 2], F32, name="gstat", tag="gstat", bufs=2)
    nc.vector.tensor_scalar_mul(gstat, gsum, 1.0 / float(GSIZE * T))
    m2 = pp.tile([G, 1], F32, name="m2", tag="m2", bufs=2)
    nc.vector.tensor_tensor(out=m2, in0=gstat[:, 0:1], in1=gstat[:, 0:1], op=ALU.mult)
    var = pp.tile([G, 1], F32, name="var", tag="var", bufs=2)
    nc.vector.tensor_tensor(out=var, in0=gstat[:, 1:2], in1=m2, op=ALU.subtract)
    nc.vector.tensor_scalar_add(var, var, EPS)
    # inv_std = rsqrt(var) via the magic-constant seed + 2 Newton steps, all
    # on DVE int/float ALU ops ([G,1] tiles - sub-100ns each). Keeps the ACT
    # engine exp-only (no per-iteration activation-table reloads).
    ish = pp.tile([G, 1], I32, name="ish", tag="ish", bufs=2)
    nc.vector.tensor_scalar(
        out=ish,
        in0=var.bitcast(I32),
        scalar1=1,
        scalar2=None,
        op0=ALU.logical_shift_right,
    )
    imag = pp.tile([G, 1], I32, name="imag", tag="imag", bufs=2)
    nc.vector.tensor_scalar(
        out=imag, in0=ish, scalar1=-1, scalar2=0x5F3759DF, op0=ALU.mult, op1=ALU.add
    )
    y = imag.bitcast(F32)
    grs = pp.tile([G, 2], F32, name="grs", tag="grs", bufs=2)
    nc.vector.tensor_copy(out=grs[:, 0:1], in_=gstat[:, 0:1])
    for it in range(2):
        vy = wp.tile([G, 1], F32, name="vy", tag="vy", bufs=2)
        nc.vector.tensor_tensor(out=vy, in0=var, in1=y, op=ALU.mult)
        vyy = wp.tile([G, 1], F32, name="vyy", tag="vyy", bufs=2)
        nc.vector.tensor_tensor(out=vyy, in0=vy, in1=y, op=ALU.mult)
        w_t = wp.tile([G, 1], F32, name="wns", tag="wns", bufs=2)
        nc.vector.tensor_scalar(
            out=w_t, in0=vyy, scalar1=-0.5, scalar2=1.5, op0=ALU.mult, op1=ALU.add
        )
        dst = grs[:, 1:2] if it == 1 else pp.tile([G, 1], F32, name="y1", tag="y1", bufs=2)
        nc.vector.tensor_tensor(out=dst, in0=y, in1=w_t, op=ALU.mult)
        y = dst

    xn8 = [
        pp.tile([128, 2, T], F8, name=f"xn8_{m}", tag=f"xn8_{m}", bufs=2)
        for m in range(2)
    ]
    xn = []
    for j in range(4):
        chs = pool_ps.tile([128, 2], F32, name=f"chs{j}", tag="sm", bufs=2)
        nc.tensor.matmul(out=chs, lhsT=gmatT[:, j], rhs=grs, start=True, stop=True)
        a_j = pp.tile([128, 1], F32, name=f"a{j}", tag=f"a{j}", bufs=2)
        nc.vector.tensor_tensor(out=a_j, in0=gnw[j], in1=chs[:, 1:2], op=ALU.mult)
        nb = wp.tile([128, 1], F32, name="nb", tag="nb")
        nc.vector.tensor_tensor(out=nb, in0=chs[:, 0:1], in1=a_j, op=ALU.mult)
        b_j = pp.tile([128, 1], F32, name=f"b{j}", tag=f"b{j}", bufs=2)
        nc.vector.tensor_tensor(out=b_j, in0=gnb[j], in1=nb, op=ALU.subtract)
        xn_j = pp.tile([128, T], BF16, name=f"xn{j}", tag=f"xn{j}", bufs=2)
        nc.vector.tensor_scalar(
            out=xn_j, in0=xt[j], scalar1=a_j, scalar2=b_j, op0=ALU.mult, op1=ALU.add
        )
        xn.append(xn_j)
        nc.vector.tensor_scalar(
            out=xn8[j // 2][:, j % 2, :],
            in0=xt[j],
            scalar1=a_j,
            scalar2=b_j,
            op0=ALU.mult,
            op1=ALU.add,
        )
    fr["xn"] = xn
    fr["xn8"] = xn8


def _emit_prologue(nc, pp, wp, pool_ps, fr):
    """Pair-0 QK + r for front `fr` (hoisted into the previous body so the
    first S matmuls of the next iteration are ready the moment its body
    starts)."""
    xn, wqkT, bq = fr["xn"], fr["wqkT"], fr["bq"]
    q0 = pp.tile([128, T], BF16, name="q0", tag="q0")
    k0 = pp.tile([128, T], BF16, name="k0", tag="k0")
    for grp in range(4):
        which, tb = grp // 2, grp % 2
        col0 = which * C
        ps = pool_ps.tile([128, 512], F32, name="qkps", tag="sm", bufs=2)
        for c in range(4):
            nc.tensor.matmul(
                out=ps,
                lhsT=wqkT[c][:, col0 : col0 + 128],
                rhs=xn[c][:, tb * 512 : (tb + 1) * 512],
                start=(c == 0),
                stop=(c == 3),
            )
        if which == 0:
            nc.vector.tensor_scalar_add(
                q0[:, tb * 512 : (tb + 1) * 512], ps, bq[:, 0:1]
            )
        else:
            nc.vector.tensor_copy(out=k0[:, tb * 512 : (tb + 1) * 512], in_=ps)
    return {"q0": q0, "k0": k0}


def _emit_attn(nc, pp, wp, pool_ps, dram, fr, pro, hook1=None, hook2=None, hook3=None):
    """Attention + projection + residual for a prepared front `fr` whose
    pair-0 QK prologue `pro` was already emitted.

    hook1/hook2 are invoked after the pair-0 / pair-2 phases to emit the next
    iteration's loads+stats and normalization; hook3 after the pair-3 S loop
    to emit the next iteration's prologue - so every engine sees the next
    front's work well before this iteration's tail drains."""
    out_r = dram["out"].rearrange("(j p) t -> j p t", p=128)
    xt, xn, xn8, bq = fr["xt"], fr["xn"], fr["xn8"], fr["bq"]
    wqkT, wv8, wproj8 = fr["wqkT"], fr["wv8"], fr["wproj8"]
    eb = fr["eb"]
    bproj = [fr["consts"][:, j, 2:3] for j in range(4)]

    q_sb = [pro["q0"], None, None, None]
    k_sb = [pro["k0"], None, None, None]
    hn_dt = BF16 if AB.get("proj_bf16") else F8
    hn_pair = [
        pp.tile([128, 2, T], hn_dt, name=f"hn{m}", tag=f"hn{m}") for m in range(2)
    ]
    vT = [None] * 4  # s-chunk pair tiles [128, 2, HEADS, HDP]

    def emit_vt_chunk(s: int) -> None:
        """V^T s-tile (fp8, DoubleRow pair layout): chunk s into pair tile
        u=s//2 slot i=s%2; 2 DoubleRow matmuls contract all 512 channels."""
        u, i = s // 2, s % 2
        vdt = BF16 if AB.get("av_bf16") else F8
        if i == 0:
            vt_u = pp.tile([128, 2, HEADS, HDP], vdt, name=f"vT{u}", tag=f"vT{u}")
            nc.gpsimd.memset(vt_u[:, :, :, HD : HD + 1], 1.0)
            vT[u] = vt_u
        vps = pool_ps.tile([128, C], F32, name=f"vps{s}", tag="sm", bufs=2)
        if AB.get("v_bf16"):
            for c in range(4):
                nc.tensor.matmul(
                    out=vps,
                    lhsT=xn[c][:, s * 128 : (s + 1) * 128],
                    rhs=wv8[c][:, 0:C],
                    start=(c == 0),
                    stop=(c == 3),
                )
            vscale = 1.0
        else:
            for m in range(2):
                nc.tensor.matmul(
                    out=vps,
                    lhsT=xn8[m][:, :, s * 128 : (s + 1) * 128],
                    rhs=wv8[m][:, :, 0:C],
                    start=(m == 0),
                    stop=(m == 1),
                    perf_mode=DR,
                )
            vscale = WINV
        nc.vector.tensor_scalar_mul(
            vT[u][:, i, :, 0:HD], vps.rearrange("p (h d) -> p h d", d=HD), vscale
        )

    def make_qk_chunks(jt: int):
        """QK o-tile pair jt as 8 chunks of 1 DoubleRow matmul each."""
        dsts = {}
        for which in range(2):
            dsts[which] = pp.tile(
                [128, T], BF16, name=f"{'qk'[which]}{jt}", tag=f"{'qk'[which]}{jt}"
            )
        state = {}

        def chunk(s: int) -> None:
            grp = s // 2  # 0..3: (which, tb)
            which, tb = grp // 2, grp % 2
            col0 = which * C + jt * 128
            if s % 2 == 0:
                state["ps"] = pool_ps.tile([128, 512], F32, name="qkps", tag="sm", bufs=2)
            ps = state["ps"]
            for c in (2 * (s % 2), 2 * (s % 2) + 1):
                nc.tensor.matmul(
                    out=ps,
                    lhsT=wqkT[c][:, col0 : col0 + 128],
                    rhs=xn[c][:, tb * 512 : (tb + 1) * 512],
                    start=(c == 0),
                    stop=(c == 3),
                )
            if s % 2 == 1:
                if which == 0:
                    nc.vector.tensor_scalar_add(
                        dsts[0][:, tb * 512 : (tb + 1) * 512], ps, bq[:, jt : jt + 1]
                    )
                else:
                    nc.vector.tensor_copy(
                        out=dsts[1][:, tb * 512 : (tb + 1) * 512], in_=ps
                    )

        def finish():
            q_sb[jt] = dsts[0]
            k_sb[jt] = dsts[1]

        return chunk, finish

    def emit_s_exp(p: int, s: int, expS) -> None:
        """S^T matmuls (bf16) + fp8 exp for head pair p, s-block s.

        The four matmuls alternate head halves (row groups 0-1 vs 2-3) so
        each LDWEIGHTS targets the array half the in-flight matmul is not
        using and the two heads' matmuls overlap on disjoint row groups."""
        jt = p
        u, i = s // 2, s % 2
        sps = {}
        for hh in range(2):
            sps[hh] = pool_ps.tile([128, T], F32, name="sps", tag="st", bufs=2)
            if i == 0:
                expS[hh].append(
                    wp.tile(
                        [128, 2, T],
                        BF16 if AB.get("av_bf16") else F8,
                        name="expS", tag="expS", bufs=16,
                    )
                )
        for hh, tb in [(0, 0), (1, 0), (0, 1), (1, 1)]:
            off = 64 * hh
            nc.tensor.matmul(
                out=sps[hh][:, tb * 512 : (tb + 1) * 512],
                lhsT=k_sb[jt][off : off + 64, s * 128 : (s + 1) * 128],
                rhs=q_sb[jt][off : off + 64, tb * 512 : (tb + 1) * 512],
                start=True,
                stop=True,
            )
        for hh in range(2):
            nc.scalar.activation(
                out=expS[hh][u][:, i, :],
                in_=sps[hh],
                func=ACTF.Exp,
                bias=eb[:, 0:1],
            )

    def make_av_chunks(p: int, expS):
        """AV (fp8 DoubleRow, K=256 per matmul) + normalize for head pair p
        as 8 chunks of 2 matmuls."""
        state = {}
        m_, i2 = p // 2, p % 2

        def chunk(s: int) -> None:
            grp = s // 2  # (hh, tb)
            hh, tb = grp // 2, grp % 2
            h = 2 * p + hh
            half = s % 2
            if half == 0:
                state["ps"] = pool_ps.tile(
                    [HD + 1, 512], F32, name="hps", tag="hp", bufs=2
                )
            hps = state["ps"]
            if AB.get("av_bf16"):
                for si in (4 * half, 4 * half + 1, 4 * half + 2, 4 * half + 3):
                    nc.tensor.matmul(
                        out=hps,
                        lhsT=vT[si // 2][:, si % 2, h, 0 : HD + 1],
                        rhs=expS[hh][si // 2][:, si % 2, tb * 512 : (tb + 1) * 512],
                        start=(si == 0),
                        stop=(si == 7),
                    )
            else:
                for u in (2 * half, 2 * half + 1):
                    nc.tensor.matmul(
                        out=hps,
                        lhsT=vT[u][:, :, h, 0 : HD + 1],
                        rhs=expS[hh][u][:, :, tb * 512 : (tb + 1) * 512],
                        start=(u == 0),
                        stop=(u == 3),
                        perf_mode=DR,
                    )
            if half == 1:
                rrow = wp.tile([1, 512], F32, name="rrow", tag="rrow", bufs=2)
                # reciprocal_approx_fast produces garbage on HW under this
                # runtime (sim-only custom DVE op path) - use the exact op.
                nc.vector.reciprocal(out=rrow, in_=hps[HD : HD + 1, :])
                rb = wp.tile([64, 512], F32, name="rb", tag="rb", bufs=2)
                nc.gpsimd.partition_broadcast(out_ap=rb, in_ap=rrow, channels=64)
                if hh == 0:
                    nc.vector.tensor_tensor(
                        out=hn_pair[m_][0:64, i2, tb * 512 : (tb + 1) * 512],
                        in0=hps[0:HD, :],
                        in1=rb,
                        op=ALU.mult,
                    )
                else:
                    hstg = wp.tile([64, 512], hn_dt, name="hstg", tag="hstg", bufs=2)
                    nc.vector.tensor_tensor(
                        out=hstg, in0=hps[0:HD, :], in1=rb, op=ALU.mult
                    )
                    nc.gpsimd.dma_start(
                        out=hn_pair[m_][64:128, i2, tb * 512 : (tb + 1) * 512],
                        in_=hstg,
                    )

        return chunk

    av_chunk = None
    for p in range(4):
        expS = {0: [], 1: []}
        if p < 3:
            qk_chunk, qk_finish = make_qk_chunks(p + 1)
        else:
            qk_chunk, qk_finish = None, None
        for s in range(8):
            emit_s_exp(p, s, expS)
            if p == 0:
                emit_vt_chunk(s)
            if av_chunk is not None:
                av_chunk(s)
            if qk_chunk is not None:
                qk_chunk(s)
        if qk_finish is not None:
            qk_finish()
        av_chunk = make_av_chunks(p, expS)
        if p == 0 and hook1 is not None:
            hook1()
        if p == 2 and hook2 is not None:
            hook2()
    if hook3 is not None:
        hook3()
    for s in range(8):
        av_chunk(s)

    # ---- projection (fp8 DoubleRow) + bias + residual ----
    for o in range(4):
        xres = wp.tile([128, T], F32, name="xres", tag="xres", bufs=2)
        nc.vector.tensor_scalar(
            out=xres, in0=xt[o], scalar1=bproj[o], scalar2=None, op0=ALU.add
        )
        oto = wp.tile([128, T], F32, name="oto", tag="oto", bufs=2)
        for tb in range(2):
            pps = pool_ps.tile([128, 512], F32, name="pps", tag="hp", bufs=2)
            if AB.get("proj_bf16"):
                for c in range(4):
                    nc.tensor.matmul(
                        out=pps,
                        lhsT=wproj8[c][:, o * 128 : (o + 1) * 128],
                        rhs=hn_pair[c // 2][:, c % 2, tb * 512 : (tb + 1) * 512],
                        start=(c == 0),
                        stop=(c == 3),
                    )
                pscale = 1.0
            else:
                for m in range(2):
                    nc.tensor.matmul(
                        out=pps,
                        lhsT=wproj8[m][:, :, o * 128 : (o + 1) * 128],
                        rhs=hn_pair[m][:, :, tb * 512 : (tb + 1) * 512],
                        start=(m == 0),
                        stop=(m == 1),
                        perf_mode=DR,
                    )
                pscale = WINV
            nc.vector.scalar_tensor_tensor(
                out=oto[:, tb * 512 : (tb + 1) * 512],
                in0=pps,
                scalar=pscale,
                in1=xres[:, tb * 512 : (tb + 1) * 512],
                op0=ALU.mult,
                op1=ALU.add,
            )
        (nc.sync if o < 2 else nc.gpsimd).dma_start(out=out_r[o], in_=oto)


def _emit_iters(nc, pp, wp, pool_ps, dram, repeats: int, w=None) -> None:
    if w is None:
        w = _emit_weights(nc, pp, dram)
    fr = _emit_front_loads(nc, pp, wp, pool_ps, dram, w)
    _emit_front_norm(nc, pp, wp, pool_ps, fr)
    pro = _emit_prologue(nc, pp, wp, pool_ps, fr)
    for i in range(repeats):
        nxt = {}
        if i < repeats - 1:
            def hook1():
                nxt["fr"] = _emit_front_loads(nc, pp, wp, pool_ps, dram, w)

            def hook2():
                _emit_front_norm(nc, pp, wp, pool_ps, nxt["fr"])

            def hook3():
                nxt["pro"] = _emit_prologue(nc, pp, wp, pool_ps, nxt["fr"])
        else:
            hook1 = hook2 = hook3 = None
        _emit_attn(nc, pp, wp, pool_ps, dram, fr, pro, hook1, hook2, hook3)
        if i < repeats - 1:
            fr, pro = nxt["fr"], nxt["pro"]


def _emit(nc, repeats: int = 1, loop_n: int | None = None) -> None:
    dram = {
        "x": nc.dram_tensor("x", [C, T], F32, kind="ExternalInput").ap(),
        "wqkT": nc.dram_tensor("wqkT", [C, 2 * C], BF16, kind="ExternalInput").ap(),
        "wv8": nc.dram_tensor("wv8", [2, 128, 2, C], F8, kind="ExternalInput").ap(),
        "wvT": nc.dram_tensor("wvT", [C, C], BF16, kind="ExternalInput").ap(),
        "bq": nc.dram_tensor("bq", [C, 1], F32, kind="ExternalInput").ap(),
        "wproj8": nc.dram_tensor("wproj8", [2, 128, 2, C], F8, kind="ExternalInput").ap(),
        "wprojT": nc.dram_tensor("wprojT", [C, C], BF16, kind="ExternalInput").ap(),
        "consts": nc.dram_tensor(
            "consts", [128, 4, NCONST], F32, kind="ExternalInput"
        ).ap(),
        "gmatT": nc.dram_tensor("gmatT", [G, 4, 128], F32, kind="ExternalInput").ap(),
        "out": nc.dram_tensor("out", [C, T], F32, kind="ExternalOutput").ap(),
    }
    with tile.TileContext(nc) as tc:
        with (
            tc.tile_pool(name="persist", bufs=1) as pp,
            tc.tile_pool(name="work", bufs=2) as wp,
            tc.tile_pool(name="psum", bufs=1, space="PSUM") as pool_ps,
        ):
            if loop_n is not None:
                w = _emit_weights(nc, pp, dram)
                with tc.For_i(0, loop_n) as _i:
                    _emit_iters(nc, pp, wp, pool_ps, dram, repeats, w=w)
            else:
                _emit_iters(nc, pp, wp, pool_ps, dram, repeats)


_NC_CACHE = {}


def build_nc(repeats: int = 1, loop_n: int | None = None):
    key = (repeats, loop_n, tuple(sorted(AB.items())))
    if key not in _NC_CACHE:
        nc = bacc.Bacc("TRN2", target_bir_lowering=False, debug=False, num_devices=B)
        _emit(nc, repeats=repeats, loop_n=loop_n)
        nc.compile()
        _NC_CACHE[key] = nc
    return _NC_CACHE[key]


def prep_inputs(x, gn_w, gn_b, qkv_w, qkv_b, proj_w, proj_b):
    """Host-side reformat: returns the per-core in_map dicts (core i = batch i)."""
    import ml_dtypes

    x = np.ascontiguousarray(np.asarray(x, dtype=np.float32))
    gn_w = np.asarray(gn_w, dtype=np.float32)
    gn_b = np.asarray(gn_b, dtype=np.float32)
    qkv_w = np.asarray(qkv_w, dtype=np.float32)
    qkv_b = np.asarray(qkv_b, dtype=np.float32)
    proj_w = np.asarray(proj_w, dtype=np.float32)
    proj_b = np.asarray(proj_b, dtype=np.float32)

    scale = float(HD) ** -0.25
    idx_q = np.concatenate([np.arange(3 * HD * h, 3 * HD * h + HD) for h in range(HEADS)])
    idx_k = idx_q + HD
    idx_v = idx_q + 2 * HD
    wq = qkv_w[idx_q] * scale
    wk = qkv_w[idx_k] * scale
    wv = qkv_w[idx_v]
    wqkT = np.ascontiguousarray(
        np.concatenate([wq, wk], axis=0).T.astype(ml_dtypes.bfloat16)
    )  # (512, 1024) bf16
    # DoubleRow pair layout: wv8[m, p, i, o] = wvT[256m + 128i + p, o] * 64
    wvT = wv.T * WSCALE  # (512, 512)
    wv8 = np.ascontiguousarray(
        wvT.reshape(2, 2, 128, C).transpose(0, 2, 1, 3).astype(ml_dtypes.float8_e4m3)
    )
    wprojT = proj_w.T * WSCALE  # (512, 512)
    wproj8 = np.ascontiguousarray(
        wprojT.reshape(2, 2, 128, C).transpose(0, 2, 1, 3).astype(ml_dtypes.float8_e4m3)
    )

    # Softmax bias algebra: K's bias contributes a per-query constant that
    # cancels in the softmax normalizer, and V's bias commutes with the
    # softmax average (weights sum to 1) -> only Q's bias is applied (on the
    # q tiles); K/V biases and the projection bias are preadded to x here.
    bq = (qkv_b[idx_q] * scale).reshape(C, 1)
    bprojK = proj_b + proj_w @ qkv_b[idx_v]

    consts = np.zeros((128, 4, NCONST), dtype=np.float32)
    gmatT = np.zeros((G, 4, 128), dtype=np.float32)
    for j in range(4):
        consts[:, j, 0] = gn_w[j * 128 : (j + 1) * 128]
        consts[:, j, 1] = gn_b[j * 128 : (j + 1) * 128]
        consts[:, j, 2] = bprojK[j * 128 : (j + 1) * 128]
        for cl in range(128):
            g = 8 * j + cl // GSIZE
            consts[cl, j, 3 + g] = 1.0  # gmat one-hot [128, G]
            gmatT[g, j, cl] = 1.0

    shared = {
        "wqkT": wqkT,
        "wv8": wv8,
        "wvT": np.ascontiguousarray(wv.T.astype(ml_dtypes.bfloat16)),
        "wprojT": np.ascontiguousarray(proj_w.T.astype(ml_dtypes.bfloat16)),
        "bq": np.ascontiguousarray(bq),
        "wproj8": wproj8,
        "consts": consts,
        "gmatT": gmatT,
    }
    in_maps = []
    for b in range(B):
        m = {"x": np.ascontiguousarray(x[b].reshape(C, T))}
        m.update(shared)
        in_maps.append(m)
    return in_maps


def kernel(x, gn_w, gn_b, qkv_w, qkv_b, proj_w, proj_b):
    import os

    # The axon client has no NTFF hook; a stray BASS_TRACE=1 would crash the
    # trace path inside run_bass_kernel_spmd.
    os.environ.setdefault("BASS_NEVER_TRACE", "1")
    in_maps = prep_inputs(x, gn_w, gn_b, qkv_w, qkv_b, proj_w, proj_b)
    nc = build_nc()
    res = run_bass_kernel_spmd(nc, in_maps, core_ids=list(range(B)))
    out = np.stack([res.results[i]["out"] for i in range(B)], axis=0)
    return out.reshape(B, C, 32, 32).astype(np.float32)
